# revision 66
# baseline (speedup 1.0000x reference)
"""CIN (xDeepFM Compressed Interaction Network) Bass/Tile kernel for TRN2.

Problem: X_0 [1024, 39, 64]; three CIN layers (units 128 each):
    had_i = outer(X_0, X_i) over channel dims, per (b, d)
    X_{i+1} = W_i @ had_i + b_i            (1x1 conv over channels)
    pooled_i = X_{i+1}.sum(d)
Output: concat(pooled_1..3) -> [1024, 384] fp32.

Strategy (pure data-parallel over batch, 8 cores, 128 samples each):
  * channel-major layout on chip: tensors stored [channels, b*64+d]
  * symmetric layer-1 fold: X0 (x) X0 has 780 unique channel pairs, not
    1521; a multiplicity-weighted static m-pattern (ceil((39-m)/8) rows
    per m, 115 of 117 rows used) covers them in 8 chunks instead of 13,
    with W1 folded as W1[o,h,m]+W1[o,m,h] host-side. Cuts layer-1 PE
    matmuls, DVE multiplies and slab DMA bytes by ~38% each
  * had formation: DVE tensor_mul against 128-row broadcast slabs of X_0
    rows. Layer-1 h-side slabs are host-gathered and loaded as plain fat
    contiguous DMAs; layer-2 slabs arrive by partition-broadcast DMA,
    with 3 of 10 groups per super-tile generated on the PE via one-hot
    selector matmuls dripped between conv matmuls (ScalarE evacuates;
    GPSIMD cannot read PSUM on TRN2)
  * startup: the ST0-critical tensors load via the gpsimd swdge ring
    (software descriptor generation is far cheaper than hwdge), while a
    dummy Pool op gates all non-critical statics on bc2-group-0's
    arrival so they never steal startup DMA bandwidth
  * cross-super-tile slab prefetch EARLY in the h-loop (h=4..14) via a
    dedicated 2-slot bc2pre pool for groups 0/1, keeping the FIFO
    allocation order equal to consumption order; the super-tile junction
    previously stalled ~10us on a tail DMA backlog
  * convs: PE matmuls, k-outer loop accumulating 4 x 512-wide PSUM banks
    per super-tile; layer-2 h=0 hadamard is split per 512-col chunk so
    its matmuls chain off the first x1 evacuation (junction overlap);
    emit_l3 runs at the top of each loop body so the PE chews layer-3
    work while waiting on slab DMA
  * layer 3 never materializes its conv: pooled_3 = W3 @ Gram(X0, X2);
    per 2 samples one PE transpose, then per-sample Gram matmuls; all
    layer-3 pieces drip into the next super-tile's conv loops; W3 loads
    into W2's SBUF tile after the last conv (SBUF is at capacity)
  * elementwise dtype fp16 (DVE 2x mode, ~3e-4 rel err)

Measured on 8 axon TRN2 cores: 444 us HW exec, rel err 3.0e-4
(baseline inherited at 523 us; identical dataflow first measured 1348).
"""

import os
import numpy as np

import concourse.bass as bass
import concourse.bacc as bacc
import concourse.mybir as mybir
import concourse.tile as tile
from concourse import bass_utils

# Walrus's redundant-ldweights elimination (rejects bass-emitted
# InstLdweights as of this toolchain — kept behind an off-by-default flag).
if os.environ.get("BASS_CIN_LDWOPT", "0") == "1" and not getattr(
        bass_utils.run_command, "_cin_ldwopt", False):
    _orig_run_command = bass_utils.run_command

    def _run_command_ldwopt(argv, **kw):
        if isinstance(argv, list):
            argv = ["--enable-ldw-opt=true" if a == "--enable-ldw-opt=false"
                    else a for a in argv]
        return _orig_run_command(argv, **kw)

    _run_command_ldwopt._cin_ldwopt = True
    bass_utils.run_command = _run_command_ldwopt

F32 = mybir.dt.float32
F16 = mybir.dt.float16

B, F, D, U = 1024, 39, 64, 128
NCORES = 8
BC = B // NCORES            # 128 samples per core
BD = BC * D                 # 8192 bd-columns per core
ST = 2048                   # super-tile width (DMA granularity)
NST = BD // ST              # 4
SUB = 512                   # matmul/evac sub-tile width (one PSUM bank)
NSUB = ST // SUB            # 4
SPS = SUB // D              # 8 samples per sub-tile
KG1 = 13                    # layer-1 chunks: 13 x (3 h-values x 39 m) = 117 rows

_CACHE: dict = {}


def _dtype_cfg():
    name = os.environ.get("BASS_CIN_DTYPE", "fp16")
    if name == "fp32":
        return F32, np.float32
    return F16, np.float16


def _off_cfg():
    """How many layer-2 h-groups (of 4) and layer-1 k-groups per super-tile
    are generated on the PE (ones-matmul broadcast + GpSimd/ScalarE
    evacuation) instead of DMA; plus whether the symmetric layer-1 fold
    is enabled (X0 (x) X0 is symmetric: 780 unique pairs covered by 8
    chunks of 117 rows instead of 13)."""
    offg2 = int(os.environ.get("BASS_CIN_OFFG2", "3"))
    offg1 = int(os.environ.get("BASS_CIN_OFFG1", "0"))
    sym = int(os.environ.get("BASS_CIN_SYM", "1"))
    return offg2, offg1, sym


C1 = 8                       # symmetric layer-1 chunks (8 x 117 >= 780 pairs)
KPER = 4                     # chunks per bc1 slab group (fat DMA descriptors)
NG1S = C1 // KPER            # bc1 slab groups per super-tile


def _sym_cover():
    """Multiplicity-weighted m-pattern + pair assignment for the symmetric
    layer-1 fold. Returns (M [117], H [C1, 117], used [C1, 117])."""
    count = [int(np.ceil((F - m) / C1)) for m in range(F)]
    M = list(range(F))
    for extra in range(1, max(count)):
        for m in range(F):
            if count[m] > extra:
                M.append(m)
    M = M + [0] * (117 - len(M))
    H = -np.ones((C1, 117), dtype=np.int64)
    used = np.zeros((C1, 117), dtype=bool)
    rows_of_m: dict = {}
    for p, m in enumerate(M[:115]):
        rows_of_m.setdefault(m, []).append(p)
    for m in range(F):
        hs = list(range(m, F))
        slots = [(c, p) for p in rows_of_m[m] for c in range(C1)]
        for (c, p), h in zip(slots, hs):
            H[c, p] = h
            used[c, p] = True
    return np.array(M), H, used


def _build(dt_e, offg2, offg1, sym) -> bacc.Bacc:
    nc = bacc.Bacc("TRN2", target_bir_lowering=False, debug=False,
                   enable_asserts=False)
    AF = mybir.ActivationFunctionType

    nk1 = C1 if sym else KG1                 # layer-1 chunk count
    kper = KPER if sym else 4                # chunks per bc1 slab group

    es1_d = None
    if sym:
        # x0mp rows: multiplicity-weighted m-pattern (rows 0..38 = X0 direct)
        x0cp_d = nc.dram_tensor("x0mp", [117, BD], dt_e, kind="ExternalInput")
        x0q1_d = nc.dram_tensor("x0q1s", [NST, NG1S, 117, KPER, ST], dt_e,
                                kind="ExternalInput")
        if offg1 > 0:
            es1_d = nc.dram_tensor("es1s", [F, C1 * 117], dt_e,
                                   kind="ExternalInput")
        w1p_d = nc.dram_tensor("w1ps", [117, C1 * U], dt_e,
                               kind="ExternalInput")
    else:
        x0cp_d = nc.dram_tensor("x0cp", [F, BD], dt_e, kind="ExternalInput")
        x0q1_d = nc.dram_tensor("x0q1", [NST, 3, KG1, ST], dt_e,
                                kind="ExternalInput")
        es1_d = nc.dram_tensor("esel1", [F, KG1 * 117], dt_e,
                               kind="ExternalInput")
        w1p_d = nc.dram_tensor("w1p", [117, KG1 * U], dt_e,
                               kind="ExternalInput")
    x0q2_d = nc.dram_tensor("x0q2", [NST, F, ST], dt_e, kind="ExternalInput")
    es2_d = nc.dram_tensor("esel2", [F, F * U], dt_e, kind="ExternalInput")
    x0dt_d = nc.dram_tensor("x0dt", [D, BC * F], dt_e, kind="ExternalInput")
    w2p_d = nc.dram_tensor("w2p", [U, F * U], dt_e, kind="ExternalInput")
    w3p_d = nc.dram_tensor("w3p", [U, F * U], dt_e, kind="ExternalInput")
    b1_d = nc.dram_tensor("b1c", [U, 1], F32, kind="ExternalInput")
    b2_d = nc.dram_tensor("b2c", [U, 1], F32, kind="ExternalInput")
    b3_d = nc.dram_tensor("b3c", [U, 1], F32, kind="ExternalInput")  # 64*b3
    id16_d = nc.dram_tensor("id16", [U, U], dt_e, kind="ExternalInput")
    id32_d = nc.dram_tensor("id32", [U, U], F32, kind="ExternalInput")

    y_d = nc.dram_tensor("y", [BC, 3 * U], F32, kind="ExternalOutput")

    ng1 = (nk1 + kper - 1) // kper           # bc1 groups per super-tile
    ng2 = (F + 3) // 4                       # 10 h-groups (9x4 + 3)
    pe1 = set(list(range(1, ng1, 2))[:offg1])
    pe2 = set(list(range(1, ng2, 2))[:offg2])
    # (measured slower: PE-generating ST0's first bc1 group regressed
    # 444us -> 458us, so it stays disabled)
    pe1_st0: set = set()

    with tile.TileContext(nc) as tc:
        with (
            tc.tile_pool(name="static", bufs=1) as stat,
            tc.tile_pool(name="bc1", bufs=2) as bc1p,
            tc.tile_pool(name="bc2pre", bufs=2) as bc2pre,
            tc.tile_pool(name="bc2", bufs=3) as bc2p,
            tc.tile_pool(name="had", bufs=3) as hadp,
            tc.tile_pool(name="xsb", bufs=1) as xsbp,
            tc.tile_pool(name="l3sb", bufs=2) as l3p,
            tc.tile_pool(name="ps_conv", bufs=4, space="PSUM") as ppc,
            tc.tile_pool(name="ps_tg", bufs=2, space="PSUM") as pptg,
            tc.tile_pool(name="ps_bc", bufs=2, space="PSUM") as ppbc,
        ):
            # ---- ST0-critical loads via the gpsimd swdge ring: descriptor
            # generation there is ~100x cheaper than hwdge (~0.34ns vs
            # ~50ns/desc), so x0st3 lands in ~8us instead of ~35us ----
            x0st3 = stat.tile([117, BD], dt_e)
            if sym:
                nc.gpsimd.dma_start(x0st3[:], x0cp_d[:, :])
            else:
                for j in range(3):
                    nc.gpsimd.dma_start(x0st3[j * F:(j + 1) * F, :],
                                        x0cp_d[:, :])
            w1sb = stat.tile([117, nk1 * U], dt_e)
            nc.gpsimd.dma_start(w1sb[:], w1p_d[:, :])
            b1sb = stat.tile([U, 1], F32)
            nc.gpsimd.dma_start(b1sb[:], b1_d[:, :])
            esel1 = None
            if es1_d is not None:
                esel1 = stat.tile([F, nk1 * 117], dt_e)
                nc.gpsimd.dma_start(esel1[:], es1_d[:, :])

            # ---- global slab-group state (cross-ST prefetch) ----
            bc1g: dict = {}
            bc2g: dict = {}
            pe1_pieces: dict = {}
            pe2_pieces: dict = {}
            dma_rr = [0]

            def ensure_bc1(st, g):
                if st >= NST or g >= ng1 or (st, g) in bc1g:
                    return
                k0 = g * kper
                kcnt = min(kper, nk1 - k0)
                if g in pe1 or (st, g) in pe1_st0:
                    slab = bc1p.tile([117, kper, ST], dt_e, tag="bc1",
                                     name="bc1pe")
                    pe1_pieces[(st, g)] = [(i, sb) for i in range(kcnt)
                                           for sb in range(NSUB)]
                else:
                    slab = bc1p.tile([117, kper, ST], dt_e, tag="bc1",
                                     name="bc1s")
                    if sym:
                        # pre-gathered h-side rows: plain contiguous load.
                        # Rides the swdge ring (Pool is idle, descriptor
                        # generation ~100x cheaper than hwdge) so bc1 never
                        # occupies the hwdge generators that bc2 broadcasts
                        # need — at startup AND at super-tile tails
                        nc.gpsimd.dma_start(slab[:, 0:kcnt, :],
                                            x0q1_d[st, g])
                    else:
                        eng = nc.sync if dma_rr[0] % 2 == 0 else nc.scalar
                        for j in range(3):
                            src = x0q1_d[st, j, k0:k0 + kcnt, :] \
                                .partition_broadcast(F)
                            eng.dma_start(
                                slab[j * F:(j + 1) * F, 0:kcnt, :], src)
                        dma_rr[0] += 1
                bc1g[(st, g)] = slab

            def ensure_bc2(st, g):
                if st >= NST or g >= ng2 or (st, g) in bc2g:
                    return
                h0 = g * 4
                hcnt = min(4, F - h0)
                # groups 0/1 live in their own small pool so they can be
                # prefetched EARLY in the previous ST's h-loop (when the DMA
                # queues are idle) without breaking the main pool's
                # allocation-order-equals-consumption-order FIFO discipline
                pool = bc2pre if g < 2 else bc2p
                if g in pe2:
                    slab = pool.tile([U, 4, ST], dt_e, tag="bc2",
                                     name="bc2pe")
                    pe2_pieces[(st, g)] = [(i, sb) for i in range(hcnt)
                                           for sb in range(NSUB)]
                else:
                    slab = pool.tile([U, 4, ST], dt_e, tag="bc2",
                                     name="bc2s")
                    src = x0q2_d[st, h0:h0 + hcnt, :].partition_broadcast(U)
                    eng = nc.sync if dma_rr[0] % 2 == 0 else nc.scalar
                    dma_rr[0] += 1
                    eng.dma_start(slab[:, 0:hcnt, :], src)
                bc2g[(st, g)] = slab

            ensure_bc1(0, 0)
            ensure_bc1(0, 1)
            ensure_bc2(0, 0)
            ensure_bc2(0, 1)

            # ---- gated loads (gpsimd swdge ring): a dummy Pool op that
            # consumes bc2 group 0 delays every subsequent swdge DMA until
            # the ST0-critical tensors have landed, so they never steal DMA
            # bandwidth from the critical startup path ----
            gate = stat.tile([1, 8], dt_e)
            nc.gpsimd.tensor_copy(gate[:], bc2g[(0, 0)][0:1, 0, 0:8])
            w2sb = stat.tile([U, F * U], dt_e)
            nc.gpsimd.dma_start(w2sb[:], w2p_d[:, :])
            esel2 = stat.tile([F, F * U], dt_e)
            nc.gpsimd.dma_start(esel2[:], es2_d[:, :])
            # two stacked copies (partitions 0-63 and 64-127) so per-sample
            # Gram matmuls can match lhsT base_partition for both halves
            x0dt = stat.tile([2 * D, BC * F], dt_e)
            nc.gpsimd.dma_start(x0dt[0:D, :], x0dt_d[:, :])
            nc.gpsimd.dma_start(x0dt[D:2 * D, :], x0dt_d[:, :])
            id16 = stat.tile([U, U], dt_e)
            nc.gpsimd.dma_start(id16[:], id16_d[:, :])
            # w3 reuses w2's tile: its load is issued after the last layer-2
            # conv consumes w2 (see end of the main loop)
            w3sb = w2sb
            b2sb = stat.tile([U, 1], F32)
            nc.gpsimd.dma_start(b2sb[:], b2_d[:, :])
            b3sb = stat.tile([U, 1], F32)
            nc.gpsimd.dma_start(b3sb[:], b3_d[:, :])
            id32 = stat.tile([U, U], F32)
            nc.gpsimd.dma_start(id32[:], id32_d[:, :])

            pooled1 = stat.tile([U, BC], F32)
            pooled2 = stat.tile([U, BC], F32)
            pooled3 = stat.tile([U, BC], F32)
            g2f = stat.tile([U, F, BC], dt_e)           # Gram: [m, h, b]
            outsb = stat.tile([BC, 3 * U], F32)

            # ---- main loop over super-tiles ----
            l3_pending: list = []

            def emit_l3():
                if l3_pending:
                    l3_pending.pop(0)()

            def gen_slab_pe2(st, slab, h, i, sb_i):
                # slab[:, i, sub] <- broadcast of X0[h, sub-cols] to 128
                # rows via selector matmul: esel2[:, h-block].T @ x0block.
                # Evacuations alternate ScalarE/GpSimd so neither FIFO
                # gates the PE through the 2-slot bps pool.
                c0 = st * ST + sb_i * SUB
                bps = ppbc.tile([U, SUB], F32, tag="bcps", name="bps")
                nc.tensor.matmul(
                    bps[:], esel2[:, h * U:(h + 1) * U],
                    x0st3[0:F, c0:c0 + SUB],
                    start=True, stop=True,
                )
                # GPSIMD cannot read PSUM on TRN2: all casts go to ScalarE
                # (Act has headroom; DVE stays exclusively on had-muls)
                dst = slab[:, i, sb_i * SUB:(sb_i + 1) * SUB]
                nc.scalar.activation(dst, bps[:], AF.Identity)

            def gen_slab_pe1(st, slab, k, i, sb_i):
                # one selector matmul builds the whole 117-row chunk:
                # out[p] = X0[3k + p//39]
                c0 = st * ST + sb_i * SUB
                bps = ppbc.tile([117, SUB], F32, tag="bcps", name="bps1")
                nc.tensor.matmul(
                    bps[:], esel1[:, k * 117:(k + 1) * 117],
                    x0st3[0:F, c0:c0 + SUB],
                    start=True, stop=True,
                )
                dst = slab[:, i, sb_i * SUB:(sb_i + 1) * SUB]
                nc.scalar.activation(dst, bps[:], AF.Identity)

            def drip_pe2_pieces(budget):
                for (gst, g) in sorted(pe2_pieces):
                    lst = pe2_pieces[(gst, g)]
                    if lst:
                        for _ in range(budget):
                            if not lst:
                                break
                            i, sb_i2 = lst.pop(0)
                            gen_slab_pe2(gst, bc2g[(gst, g)], g * 4 + i,
                                         i, sb_i2)
                        return True
                return False

            def bc1_drip(st, k):
                ensure_bc1(st, k // kper + 2)
                for (gst, g) in sorted(pe1_pieces):
                    lst = pe1_pieces[(gst, g)]
                    if lst:
                        for _ in range(4):
                            if not lst:
                                break
                            i, sb_i2 = lst.pop(0)
                            gen_slab_pe1(gst, bc1g[(gst, g)], g * kper + i,
                                         i, sb_i2)
                        return
                # no bc1 pieces pending: pre-build pending bc2 pieces
                # (not on ST0, where esel2 is still loading — an early piece
                # matmul would head-of-line-block the in-order PE stream)
                if st > 0:
                    drip_pe2_pieces(3)

            def bc2_drip(st, h):
                ensure_bc2(st, h // 4 + 2)
                # cross-ST prefetch EARLY in the h-loop, when the DMA queues
                # are idle (issuing in the tail piles onto the backlog of
                # this ST's own late bc2 groups and stalls the junction)
                if h == 4:
                    ensure_bc1(st + 1, 0)
                elif h == 6:
                    ensure_bc1(st + 1, 1)
                elif h == 8:
                    ensure_bc2(st + 1, 0)
                elif h == 10:
                    ensure_bc2(st + 1, 1)
                elif h == 30:
                    ensure_bc2(st + 1, 2)
                if h >= F - 3:
                    # keep the Act queue shallow at the super-tile junction
                    # (pending casts would delay the x2 PSUM evacuations);
                    # deferred pieces drip during the next ST's loops
                    return
                drip_pe2_pieces(4)

            # eagerly build the first slab chunk of the PE-generated ST0
            # group so k=0's hadamard has its operand as soon as x0st3 lands
            for (gst, g) in sorted(pe1_pieces):
                lst = pe1_pieces[(gst, g)]
                for _ in range(4):
                    if not lst:
                        break
                    i, sb_i2 = lst.pop(0)
                    gen_slab_pe1(gst, bc1g[(gst, g)], g * kper + i, i, sb_i2)
                break

            for st in range(NST):
                cols = slice(st * ST, (st + 1) * ST)

                # ---- layer 1: X1 = W1 @ (X0 (x) X0) + b1 ----
                x1ps = [ppc.tile([U, SUB], F32, tag="conv", name=f"x1ps{i}")
                         for i in range(NSUB)]
                for k in range(nk1):
                    # l3 work first: it has no slab dependency, so the PE
                    # chews it while waiting for this iteration's slab DMA
                    emit_l3()
                    had1 = hadp.tile([117, ST], dt_e, tag="had1")
                    nc.vector.tensor_mul(
                        had1[:], x0st3[:, cols],
                        bc1g[(st, k // kper)][:, k % kper, :])
                    for sb_i in range(NSUB):
                        nc.tensor.matmul(
                            x1ps[sb_i][:], w1sb[:, k * U:(k + 1) * U],
                            had1[:, sb_i * SUB:(sb_i + 1) * SUB],
                            start=(k == 0), stop=(k == nk1 - 1),
                        )
                    bc1_drip(st, k)
                x1sb = xsbp.tile([U, ST], dt_e, tag="x1")
                nc.scalar.activation(
                    x1sb[:, 0:SUB], x1ps[0][:],
                    AF.Identity, bias=b1sb[:], scale=1.0)
                for sb_i in range(1, NSUB):
                    nc.scalar.activation(
                        x1sb[:, sb_i * SUB:(sb_i + 1) * SUB], x1ps[sb_i][:],
                        AF.Identity, bias=b1sb[:], scale=1.0)
                # pooled1 contribution: strided d-sum on DVE, dripped later
                # so it never sits between the layer-1 evacs and layer-2 TTs
                def red1(st=st, x1sb=x1sb):
                    nc.vector.tensor_reduce(
                        pooled1[:, st * (ST // D):(st + 1) * (ST // D)],
                        x1sb[:].rearrange("p (b d) -> p b d", d=D),
                        mybir.AxisListType.X, mybir.AluOpType.add)
                l3_pending.append(red1)

                # ---- layer 2: X2 = W2 @ (X0 (x) X1) + b2 ----
                x2ps = [ppc.tile([U, SUB], F32, tag="conv", name=f"x2ps{i}")
                         for i in range(NSUB)]
                for h in range(F):
                    emit_l3()
                    had2 = hadp.tile([U, ST], dt_e, tag="had2")
                    if h == 0:
                        # split per 512-col chunk: each mul chains off its
                        # x1 sub-tile evacuation, so the first L2 matmuls
                        # start as soon as x1ps[0] is evacuated
                        for sb_i in range(NSUB):
                            cs = slice(sb_i * SUB, (sb_i + 1) * SUB)
                            nc.vector.tensor_mul(
                                had2[:, cs], x1sb[:, cs],
                                bc2g[(st, 0)][:, 0, cs])
                            nc.tensor.matmul(
                                x2ps[sb_i][:], w2sb[:, 0:U],
                                had2[:, cs],
                                start=True, stop=False,
                            )
                    else:
                        nc.vector.tensor_mul(
                            had2[:], x1sb[:], bc2g[(st, h // 4)][:, h % 4, :])
                        for sb_i in range(NSUB):
                            nc.tensor.matmul(
                                x2ps[sb_i][:], w2sb[:, h * U:(h + 1) * U],
                                had2[:, sb_i * SUB:(sb_i + 1) * SUB],
                                start=False, stop=(h == F - 1),
                            )
                    bc2_drip(st, h)
                x2sb = xsbp.tile([U, ST], dt_e, tag="x2")
                nc.scalar.activation(
                    x2sb[:, 0:SUB], x2ps[0][:],
                    AF.Identity, bias=b2sb[:], scale=1.0)
                for sb_i in range(1, NSUB):
                    nc.scalar.activation(
                        x2sb[:, sb_i * SUB:(sb_i + 1) * SUB], x2ps[sb_i][:],
                        AF.Identity, bias=b2sb[:], scale=1.0)
                def red2(st=st, x2sb=x2sb):
                    nc.vector.tensor_reduce(
                        pooled2[:, st * (ST // D):(st + 1) * (ST // D)],
                        x2sb[:].rearrange("p (b d) -> p b d", d=D),
                        mybir.AxisListType.X, mybir.AluOpType.add)
                l3_pending.append(red2)

                # ---- layer 3 Gram: G2[m, h, b] = sum_d X2[m,bd] X0[h,bd] ----
                # queue as lazily-emitted pieces, dripped into the next
                # super-tile's conv loops so the PE never starves (HAM warm)
                def queue_l3(st=st, x2sb=x2sb):
                    for s2 in range(ST // D // 2):       # 2 samples / transpose
                        def piece(s2=s2, st=st, x2sb=x2sb):
                            x2t_ps = pptg.tile([U, U], dt_e, tag="tg",
                                               name="x2tps")
                            nc.tensor.transpose(
                                x2t_ps[:],
                                x2sb[:, s2 * 2 * D:(s2 + 1) * 2 * D], id16[:])
                            x2t = l3p.tile([U, U], dt_e, tag="x2t", name="x2t")
                            nc.scalar.activation(x2t[:], x2t_ps[:], AF.Identity)
                            for ls in range(2):
                                b = st * (ST // D) + s2 * 2 + ls
                                g2ps = pptg.tile([U, F], F32, tag="tg",
                                                 name="g2ps")
                                nc.tensor.matmul(
                                    g2ps[:], x2t[ls * D:(ls + 1) * D, :],
                                    x0dt[ls * D:(ls + 1) * D,
                                         b * F:(b + 1) * F],
                                    start=True, stop=True,
                                )
                                nc.scalar.activation(
                                    g2f[:, :, b], g2ps[:], AF.Identity)
                        l3_pending.append(piece)
                queue_l3()
                if st == NST - 1:
                    # w2 is dead after the last conv: pull w3 into its tile
                    nc.gpsimd.dma_start(w3sb[:], w3p_d[:, :])
            while l3_pending:     # flush the last super-tile's layer-3 work
                l3_pending.pop(0)()

            # ---- pooled3 = W3 @ G2 + 64*b3 ----
            if True:
                p3ps = ppbc.tile([U, BC], F32, tag="bcps", name="p3ps")
                for h in range(F):
                    nc.tensor.matmul(
                        p3ps[:], w3sb[:, h * U:(h + 1) * U], g2f[:, h, :],
                        start=(h == 0), stop=(h == F - 1),
                    )
                nc.scalar.activation(
                    pooled3[:], p3ps[:], AF.Identity, bias=b3sb[:], scale=1.0)

                # ---- transpose pooled_i -> [b, o] and store ----
                for i, pl in enumerate((pooled1, pooled2, pooled3)):
                    trp = ppbc.tile([BC, U], F32, tag="bcps", name="trp")
                    nc.tensor.transpose(trp[:], pl[:], id32[:])
                    nc.scalar.activation(
                        outsb[:, i * U:(i + 1) * U], trp[:], AF.Identity)
                nc.sync.dma_start(y_d[:, :], outsb[:])

    nc.compile()
    return nc


def _prep_in_maps(inputs, np_e, sym, offg1):
    X0 = np.asarray(inputs["X_0"], np.float32)
    W1 = np.asarray(inputs["W1"], np.float32)
    b1 = np.asarray(inputs["b1"], np.float32)
    W2 = np.asarray(inputs["W2"], np.float32)
    b2 = np.asarray(inputs["b2"], np.float32)
    W3 = np.asarray(inputs["W3"], np.float32)
    b3 = np.asarray(inputs["b3"], np.float32)

    # [m, h*128+o]
    w2p = W2.reshape(U, F, U).transpose(2, 1, 0).reshape(U, F * U)
    w3p = W3.reshape(U, F, U).transpose(2, 1, 0).reshape(U, F * U)
    es2 = np.zeros((F, F * U), np.float32)
    for h in range(F):
        es2[h, h * U:(h + 1) * U] = 1.0
    es2 = es2.astype(np_e)

    w1r = W1.reshape(U, F, F)                    # [o, h, m]
    if sym:
        # symmetric fold: W1s[o,h,m] = W1[o,h,m] + W1[o,m,h] (h>m), diag as-is
        Msym, Hsym, used = _sym_cover()
        w1s = w1r + w1r.transpose(0, 2, 1)
        for h in range(F):
            w1s[:, h, h] = w1r[:, h, h]
        w1p = np.zeros((117, C1 * U), np.float32)
        es1 = np.zeros((F, C1 * 117), np.float32)
        Hfill = np.where(used, Hsym, 0)
        for k in range(C1):
            for p in range(117):
                if used[k, p]:
                    w1p[p, k * U:(k + 1) * U] = w1s[:, Hsym[k, p], Msym[p]]
                    es1[Hsym[k, p], k * 117 + p] = 1.0
    else:
        # rows p=j*39+m, cols k*128+o -> W1[o, (3k+j)*39+m]
        w1p = np.zeros((117, KG1 * U), np.float32)
        for k in range(KG1):
            for j in range(3):
                w1p[j * F:(j + 1) * F, k * U:(k + 1) * U] = \
                    w1r[:, 3 * k + j, :].T
        es1 = np.zeros((F, KG1 * 117), np.float32)
        for k in range(KG1):
            for p in range(117):
                es1[3 * k + p // F, k * 117 + p] = 1.0
    es1 = es1.astype(np_e)

    shared = {
        "w2p": w2p.astype(np_e),
        "w3p": w3p.astype(np_e),
        "b1c": b1.reshape(U, 1).astype(np.float32),
        "b2c": b2.reshape(U, 1).astype(np.float32),
        "b3c": (D * b3).reshape(U, 1).astype(np.float32),
        "id16": np.eye(U, dtype=np_e),
        "id32": np.eye(U, dtype=np.float32),
    }
    if sym:
        shared["w1ps"] = w1p.astype(np_e)
        if offg1 > 0:
            shared["es1s"] = es1
    else:
        shared["w1p"] = w1p.astype(np_e)
        shared["esel1"] = es1

    in_maps = []
    for c in range(NCORES):
        xs = X0[c * BC:(c + 1) * BC]                         # [128, 39, 64]
        x0cp = xs.transpose(1, 0, 2).reshape(F, BD)          # [h, b*64+d]
        x0dt = xs.transpose(2, 0, 1).reshape(D, BC * F)      # [d, b*39+h]
        x0st = x0cp.reshape(F, NST, ST)
        x0q2 = x0st.transpose(1, 0, 2)                       # [st, h, c]
        m = dict(shared)
        if sym:
            # h-side slab rows gathered on host: [st, g, p, i, c]
            x0q1s = x0cp[np.where(used, Hsym, 0), :]         # [C1, 117, BD]
            x0q1s = x0q1s.reshape(NG1S, KPER, 117, NST, ST)
            x0q1s = np.ascontiguousarray(x0q1s.transpose(3, 0, 2, 1, 4))
            m["x0q1s"] = x0q1s.astype(np_e)
            m["x0mp"] = np.ascontiguousarray(x0cp[Msym, :]).astype(np_e)
        else:
            x0q1 = np.zeros((NST, 3, KG1, ST), np.float32)
            for j in range(3):
                for k in range(KG1):
                    x0q1[:, j, k, :] = x0st[3 * k + j].reshape(NST, ST)
            m["x0q1"] = np.ascontiguousarray(x0q1).astype(np_e)
            m["x0cp"] = x0cp.astype(np_e)
        m["x0dt"] = x0dt.astype(np_e)
        m["x0q2"] = np.ascontiguousarray(x0q2).astype(np_e)
        m["esel2"] = es2
        in_maps.append(m)
    return in_maps


def _run(inputs, trace=False, **kw):
    dt_e, np_e = _dtype_cfg()
    offg2, offg1, sym = _off_cfg()
    key = (dt_e, offg2, offg1, sym)
    if key not in _CACHE:
        _CACHE[key] = _build(dt_e, offg2, offg1, sym)
    nc = _CACHE[key]
    in_maps = _prep_in_maps(inputs, np_e, sym, offg1)
    res = bass_utils.run_bass_kernel_spmd(
        nc, in_maps, core_ids=list(range(NCORES)), trace=trace, **kw)
    y = np.concatenate([r["y"] for r in res.results], axis=0).astype(np.float32)
    return y, res


def kernel(**inputs) -> np.ndarray:
    y, _ = _run(inputs, trace=False)
    return y


# revision 67
# speedup vs baseline: 1.0386x; 1.0386x over previous
"""CIN (xDeepFM Compressed Interaction Network) Bass/Tile kernel for TRN2.

Problem: X_0 [1024, 39, 64]; three CIN layers (units 128 each):
    had_i = outer(X_0, X_i) over channel dims, per (b, d)
    X_{i+1} = W_i @ had_i + b_i            (1x1 conv over channels)
    pooled_i = X_{i+1}.sum(d)
Output: concat(pooled_1..3) -> [1024, 384] fp32.

Strategy (pure data-parallel over batch, 8 cores, 128 samples each):
  * channel-major layout on chip: tensors stored [channels, b*64+d]
  * symmetric layer-1 fold: X0 (x) X0 has 780 unique channel pairs, not
    1521; a multiplicity-weighted static m-pattern (ceil((39-m)/8) rows
    per m, 115 of 117 rows used) covers them in 8 chunks instead of 13,
    with W1 folded as W1[o,h,m]+W1[o,m,h] host-side. Cuts layer-1 PE
    matmuls, DVE multiplies and slab DMA bytes by ~38% each
  * had formation: DVE tensor_mul against 128-row broadcast slabs of X_0
    rows. Layer-1 h-side slabs are host-gathered and loaded as plain fat
    contiguous DMAs; layer-2 slabs arrive by partition-broadcast DMA,
    with 3 of 10 groups per super-tile generated on the PE via one-hot
    selector matmuls dripped between conv matmuls (ScalarE evacuates;
    GPSIMD cannot read PSUM on TRN2)
  * startup: the ST0-critical tensors load via the gpsimd swdge ring
    (software descriptor generation is far cheaper than hwdge), while a
    dummy Pool op gates all non-critical statics on bc2-group-0's
    arrival so they never steal startup DMA bandwidth
  * cross-super-tile slab prefetch EARLY in the h-loop (h=4..14) via a
    dedicated 2-slot bc2pre pool for groups 0/1, keeping the FIFO
    allocation order equal to consumption order; the super-tile junction
    previously stalled ~10us on a tail DMA backlog
  * convs: PE matmuls, k-outer loop accumulating 4 x 512-wide PSUM banks
    per super-tile; layer-2 h=0 hadamard is split per 512-col chunk so
    its matmuls chain off the first x1 evacuation (junction overlap);
    emit_l3 runs at the top of each loop body so the PE chews layer-3
    work while waiting on slab DMA
  * layer 3 never materializes its conv: pooled_3 = W3 @ Gram(X0, X2);
    per 2 samples one PE transpose, then per-sample Gram matmuls; all
    layer-3 pieces drip into the next super-tile's conv loops; W3 loads
    into W2's SBUF tile after the last conv (SBUF is at capacity)
  * elementwise dtype fp16 (DVE 2x mode, ~3e-4 rel err)

Measured on 8 axon TRN2 cores: 444 us HW exec, rel err 3.0e-4
(baseline inherited at 523 us; identical dataflow first measured 1348).
"""

import os
import numpy as np

import concourse.bass as bass
import concourse.bacc as bacc
import concourse.mybir as mybir
import concourse.tile as tile
from concourse import bass_utils

# Walrus's redundant-ldweights elimination (rejects bass-emitted
# InstLdweights as of this toolchain — kept behind an off-by-default flag).
if os.environ.get("BASS_CIN_LDWOPT", "0") == "1" and not getattr(
        bass_utils.run_command, "_cin_ldwopt", False):
    _orig_run_command = bass_utils.run_command

    def _run_command_ldwopt(argv, **kw):
        if isinstance(argv, list):
            argv = ["--enable-ldw-opt=true" if a == "--enable-ldw-opt=false"
                    else a for a in argv]
        return _orig_run_command(argv, **kw)

    _run_command_ldwopt._cin_ldwopt = True
    bass_utils.run_command = _run_command_ldwopt

F32 = mybir.dt.float32
F16 = mybir.dt.float16

B, F, D, U = 1024, 39, 64, 128
NCORES = 8
BC = B // NCORES            # 128 samples per core
BD = BC * D                 # 8192 bd-columns per core
ST = 2048                   # super-tile width (DMA granularity)
NST = BD // ST              # 4
SUB = 512                   # matmul/evac sub-tile width (one PSUM bank)
NSUB = ST // SUB            # 4
SPS = SUB // D              # 8 samples per sub-tile
KG1 = 13                    # layer-1 chunks: 13 x (3 h-values x 39 m) = 117 rows

_CACHE: dict = {}


def _dtype_cfg():
    name = os.environ.get("BASS_CIN_DTYPE", "fp16")
    if name == "fp32":
        return F32, np.float32
    return F16, np.float16


def _off_cfg():
    """How many layer-2 h-groups (of 4) and layer-1 k-groups per super-tile
    are generated on the PE (ones-matmul broadcast + GpSimd/ScalarE
    evacuation) instead of DMA; plus whether the symmetric layer-1 fold
    is enabled (X0 (x) X0 is symmetric: 780 unique pairs covered by 8
    chunks of 117 rows instead of 13)."""
    offg2 = int(os.environ.get("BASS_CIN_OFFG2", "3"))
    offg1 = int(os.environ.get("BASS_CIN_OFFG1", "0"))
    sym = int(os.environ.get("BASS_CIN_SYM", "1"))
    return offg2, offg1, sym


C1 = 8                       # symmetric layer-1 chunks (8 x 117 >= 780 pairs)
KPER = 4                     # chunks per bc1 slab group (fat DMA descriptors)
NG1S = C1 // KPER            # bc1 slab groups per super-tile


def _sym_cover():
    """Multiplicity-weighted m-pattern + pair assignment for the symmetric
    layer-1 fold. Returns (M [117], H [C1, 117], used [C1, 117])."""
    count = [int(np.ceil((F - m) / C1)) for m in range(F)]
    M = list(range(F))
    for extra in range(1, max(count)):
        for m in range(F):
            if count[m] > extra:
                M.append(m)
    M = M + [0] * (117 - len(M))
    H = -np.ones((C1, 117), dtype=np.int64)
    used = np.zeros((C1, 117), dtype=bool)
    rows_of_m: dict = {}
    for p, m in enumerate(M[:115]):
        rows_of_m.setdefault(m, []).append(p)
    for m in range(F):
        hs = list(range(m, F))
        slots = [(c, p) for p in rows_of_m[m] for c in range(C1)]
        for (c, p), h in zip(slots, hs):
            H[c, p] = h
            used[c, p] = True
    return np.array(M), H, used


def _build(dt_e, offg2, offg1, sym) -> bacc.Bacc:
    nc = bacc.Bacc("TRN2", target_bir_lowering=False, debug=False,
                   enable_asserts=False)
    AF = mybir.ActivationFunctionType

    nk1 = C1 if sym else KG1                 # layer-1 chunk count
    kper = KPER if sym else 4                # chunks per bc1 slab group

    es1_d = None
    if sym:
        # x0mp rows: multiplicity-weighted m-pattern (rows 0..38 = X0 direct)
        x0cp_d = nc.dram_tensor("x0mp", [117, BD], dt_e, kind="ExternalInput")
        x0q1_d = nc.dram_tensor("x0q1s", [NST, NG1S, 117, KPER, ST], dt_e,
                                kind="ExternalInput")
        if offg1 > 0:
            es1_d = nc.dram_tensor("es1s", [F, C1 * 117], dt_e,
                                   kind="ExternalInput")
        w1p_d = nc.dram_tensor("w1ps", [117, C1 * U], dt_e,
                               kind="ExternalInput")
    else:
        x0cp_d = nc.dram_tensor("x0cp", [F, BD], dt_e, kind="ExternalInput")
        x0q1_d = nc.dram_tensor("x0q1", [NST, 3, KG1, ST], dt_e,
                                kind="ExternalInput")
        es1_d = nc.dram_tensor("esel1", [F, KG1 * 117], dt_e,
                               kind="ExternalInput")
        w1p_d = nc.dram_tensor("w1p", [117, KG1 * U], dt_e,
                               kind="ExternalInput")
    x0q2_d = nc.dram_tensor("x0q2", [NST, F, ST], dt_e, kind="ExternalInput")
    es2_d = nc.dram_tensor("esel2", [F, F * U], dt_e, kind="ExternalInput")
    x0dt_d = nc.dram_tensor("x0dt", [D, BC * F], dt_e, kind="ExternalInput")
    w2p_d = nc.dram_tensor("w2p", [U, F * U], dt_e, kind="ExternalInput")
    w3p_d = nc.dram_tensor("w3p", [U, F * U], dt_e, kind="ExternalInput")
    b1_d = nc.dram_tensor("b1c", [U, 1], F32, kind="ExternalInput")
    b2_d = nc.dram_tensor("b2c", [U, 1], F32, kind="ExternalInput")
    b3_d = nc.dram_tensor("b3c", [U, 1], F32, kind="ExternalInput")  # 64*b3
    id16_d = nc.dram_tensor("id16", [U, U], dt_e, kind="ExternalInput")
    id32_d = nc.dram_tensor("id32", [U, U], F32, kind="ExternalInput")

    y_d = nc.dram_tensor("y", [BC, 3 * U], F32, kind="ExternalOutput")

    ng1 = (nk1 + kper - 1) // kper           # bc1 groups per super-tile
    ng2 = (F + 3) // 4                       # 10 h-groups (9x4 + 3)
    pe1 = set(list(range(1, ng1, 2))[:offg1])
    pe2 = set(list(range(1, ng2, 2))[:offg2])
    # (measured slower: PE-generating ST0's first bc1 group regressed
    # 444us -> 458us, so it stays disabled)
    pe1_st0: set = set()

    with tile.TileContext(nc) as tc:
        with (
            tc.tile_pool(name="static", bufs=1) as stat,
            tc.tile_pool(name="bc1", bufs=2) as bc1p,
            tc.tile_pool(name="bc2pre", bufs=2) as bc2pre,
            tc.tile_pool(name="bc2", bufs=3) as bc2p,
            tc.tile_pool(name="had", bufs=3) as hadp,
            tc.tile_pool(name="xsb", bufs=1) as xsbp,
            tc.tile_pool(name="l3sb", bufs=2) as l3p,
            tc.tile_pool(name="ps_conv", bufs=4, space="PSUM") as ppc,
            tc.tile_pool(name="ps_tg", bufs=2, space="PSUM") as pptg,
            tc.tile_pool(name="ps_bc", bufs=2, space="PSUM") as ppbc,
        ):
            # ---- ST0-critical loads via the gpsimd swdge ring: descriptor
            # generation there is ~100x cheaper than hwdge (~0.34ns vs
            # ~50ns/desc), so x0st3 lands in ~8us instead of ~35us ----
            x0st3 = stat.tile([117, BD], dt_e)
            if sym:
                nc.gpsimd.dma_start(x0st3[:], x0cp_d[:, :])
            else:
                for j in range(3):
                    nc.gpsimd.dma_start(x0st3[j * F:(j + 1) * F, :],
                                        x0cp_d[:, :])
            w1sb = stat.tile([117, nk1 * U], dt_e)
            nc.gpsimd.dma_start(w1sb[:], w1p_d[:, :])
            b1sb = stat.tile([U, 1], F32)
            nc.gpsimd.dma_start(b1sb[:], b1_d[:, :])
            esel1 = None
            if es1_d is not None:
                esel1 = stat.tile([F, nk1 * 117], dt_e)
                nc.gpsimd.dma_start(esel1[:], es1_d[:, :])

            # ---- global slab-group state (cross-ST prefetch) ----
            bc1g: dict = {}
            bc2g: dict = {}
            pe1_pieces: dict = {}
            pe2_pieces: dict = {}
            dma_rr = [0]

            def ensure_bc1(st, g):
                if st >= NST or g >= ng1 or (st, g) in bc1g:
                    return
                k0 = g * kper
                kcnt = min(kper, nk1 - k0)
                if g in pe1 or (st, g) in pe1_st0:
                    slab = bc1p.tile([117, kper, ST], dt_e, tag="bc1",
                                     name="bc1pe")
                    pe1_pieces[(st, g)] = [(i, sb) for i in range(kcnt)
                                           for sb in range(NSUB)]
                else:
                    slab = bc1p.tile([117, kper, ST], dt_e, tag="bc1",
                                     name="bc1s")
                    if sym:
                        # pre-gathered h-side rows: plain contiguous load.
                        # Mid-kernel prefetches ride the swdge ring (Pool is
                        # idle; keeps bc1 off the hwdge generators that bc2
                        # broadcasts need at super-tile tails). ST0's groups
                        # use hwdge: swdge serializes the bulk startup
                        # transfers behind x0st3 (measured +11us to start)
                        if st == 0:
                            eng = nc.sync if dma_rr[0] % 2 == 0 \
                                else nc.scalar
                            dma_rr[0] += 1
                        else:
                            eng = nc.gpsimd
                        eng.dma_start(slab[:, 0:kcnt, :], x0q1_d[st, g])
                    else:
                        eng = nc.sync if dma_rr[0] % 2 == 0 else nc.scalar
                        for j in range(3):
                            src = x0q1_d[st, j, k0:k0 + kcnt, :] \
                                .partition_broadcast(F)
                            eng.dma_start(
                                slab[j * F:(j + 1) * F, 0:kcnt, :], src)
                        dma_rr[0] += 1
                bc1g[(st, g)] = slab

            def ensure_bc2(st, g):
                if st >= NST or g >= ng2 or (st, g) in bc2g:
                    return
                h0 = g * 4
                hcnt = min(4, F - h0)
                # groups 0/1 live in their own small pool so they can be
                # prefetched EARLY in the previous ST's h-loop (when the DMA
                # queues are idle) without breaking the main pool's
                # allocation-order-equals-consumption-order FIFO discipline
                pool = bc2pre if g < 2 else bc2p
                if g in pe2:
                    slab = pool.tile([U, 4, ST], dt_e, tag="bc2",
                                     name="bc2pe")
                    pe2_pieces[(st, g)] = [(i, sb) for i in range(hcnt)
                                           for sb in range(NSUB)]
                else:
                    slab = pool.tile([U, 4, ST], dt_e, tag="bc2",
                                     name="bc2s")
                    src = x0q2_d[st, h0:h0 + hcnt, :].partition_broadcast(U)
                    eng = nc.sync if dma_rr[0] % 2 == 0 else nc.scalar
                    dma_rr[0] += 1
                    eng.dma_start(slab[:, 0:hcnt, :], src)
                bc2g[(st, g)] = slab

            ensure_bc1(0, 0)
            ensure_bc1(0, 1)
            ensure_bc2(0, 0)
            ensure_bc2(0, 1)

            # ---- gated loads (gpsimd swdge ring): a dummy Pool op that
            # consumes bc2 group 0 delays every subsequent swdge DMA until
            # the ST0-critical tensors have landed, so they never steal DMA
            # bandwidth from the critical startup path ----
            gate = stat.tile([1, 8], dt_e)
            nc.gpsimd.tensor_copy(gate[:], bc2g[(0, 0)][0:1, 0, 0:8])
            w2sb = stat.tile([U, F * U], dt_e)
            nc.gpsimd.dma_start(w2sb[:], w2p_d[:, :])
            esel2 = stat.tile([F, F * U], dt_e)
            nc.gpsimd.dma_start(esel2[:], es2_d[:, :])
            # two stacked copies (partitions 0-63 and 64-127) so per-sample
            # Gram matmuls can match lhsT base_partition for both halves
            x0dt = stat.tile([2 * D, BC * F], dt_e)
            nc.gpsimd.dma_start(x0dt[0:D, :], x0dt_d[:, :])
            nc.gpsimd.dma_start(x0dt[D:2 * D, :], x0dt_d[:, :])
            id16 = stat.tile([U, U], dt_e)
            nc.gpsimd.dma_start(id16[:], id16_d[:, :])
            # w3 reuses w2's tile: its load is issued after the last layer-2
            # conv consumes w2 (see end of the main loop)
            w3sb = w2sb
            b2sb = stat.tile([U, 1], F32)
            nc.gpsimd.dma_start(b2sb[:], b2_d[:, :])
            b3sb = stat.tile([U, 1], F32)
            nc.gpsimd.dma_start(b3sb[:], b3_d[:, :])
            id32 = stat.tile([U, U], F32)
            nc.gpsimd.dma_start(id32[:], id32_d[:, :])

            pooled1 = stat.tile([U, BC], F32)
            pooled2 = stat.tile([U, BC], F32)
            pooled3 = stat.tile([U, BC], F32)
            g2f = stat.tile([U, F, BC], dt_e)           # Gram: [m, h, b]
            outsb = stat.tile([BC, 3 * U], F32)

            # ---- main loop over super-tiles ----
            l3_pending: list = []

            def emit_l3():
                if l3_pending:
                    l3_pending.pop(0)()

            def gen_slab_pe2(st, slab, h, i, sb_i):
                # slab[:, i, sub] <- broadcast of X0[h, sub-cols] to 128
                # rows via selector matmul: esel2[:, h-block].T @ x0block.
                # Evacuations alternate ScalarE/GpSimd so neither FIFO
                # gates the PE through the 2-slot bps pool.
                c0 = st * ST + sb_i * SUB
                bps = ppbc.tile([U, SUB], F32, tag="bcps", name="bps")
                nc.tensor.matmul(
                    bps[:], esel2[:, h * U:(h + 1) * U],
                    x0st3[0:F, c0:c0 + SUB],
                    start=True, stop=True,
                )
                # GPSIMD cannot read PSUM on TRN2: all casts go to ScalarE
                # (Act has headroom; DVE stays exclusively on had-muls)
                dst = slab[:, i, sb_i * SUB:(sb_i + 1) * SUB]
                nc.scalar.activation(dst, bps[:], AF.Identity)

            def gen_slab_pe1(st, slab, k, i, sb_i):
                # one selector matmul builds the whole 117-row chunk:
                # out[p] = X0[3k + p//39]
                c0 = st * ST + sb_i * SUB
                bps = ppbc.tile([117, SUB], F32, tag="bcps", name="bps1")
                nc.tensor.matmul(
                    bps[:], esel1[:, k * 117:(k + 1) * 117],
                    x0st3[0:F, c0:c0 + SUB],
                    start=True, stop=True,
                )
                dst = slab[:, i, sb_i * SUB:(sb_i + 1) * SUB]
                nc.scalar.activation(dst, bps[:], AF.Identity)

            def drip_pe2_pieces(budget):
                for (gst, g) in sorted(pe2_pieces):
                    lst = pe2_pieces[(gst, g)]
                    if lst:
                        for _ in range(budget):
                            if not lst:
                                break
                            i, sb_i2 = lst.pop(0)
                            gen_slab_pe2(gst, bc2g[(gst, g)], g * 4 + i,
                                         i, sb_i2)
                        return True
                return False

            def bc1_drip(st, k):
                ensure_bc1(st, k // kper + 2)
                for (gst, g) in sorted(pe1_pieces):
                    lst = pe1_pieces[(gst, g)]
                    if lst:
                        for _ in range(4):
                            if not lst:
                                break
                            i, sb_i2 = lst.pop(0)
                            gen_slab_pe1(gst, bc1g[(gst, g)], g * kper + i,
                                         i, sb_i2)
                        return
                # no bc1 pieces pending: pre-build pending bc2 pieces
                # (not on ST0, where esel2 is still loading — an early piece
                # matmul would head-of-line-block the in-order PE stream)
                if st > 0:
                    drip_pe2_pieces(3)

            def bc2_drip(st, h):
                ensure_bc2(st, h // 4 + 2)
                # cross-ST prefetch EARLY in the h-loop, when the DMA queues
                # are idle (issuing in the tail piles onto the backlog of
                # this ST's own late bc2 groups and stalls the junction)
                if h == 4:
                    ensure_bc1(st + 1, 0)
                elif h == 6:
                    ensure_bc1(st + 1, 1)
                elif h == 8:
                    ensure_bc2(st + 1, 0)
                elif h == 10:
                    ensure_bc2(st + 1, 1)
                elif h == 30:
                    ensure_bc2(st + 1, 2)
                if h >= F - 3:
                    # keep the Act queue shallow at the super-tile junction
                    # (pending casts would delay the x2 PSUM evacuations);
                    # deferred pieces drip during the next ST's loops
                    return
                drip_pe2_pieces(4)

            # eagerly build the first slab chunk of the PE-generated ST0
            # group so k=0's hadamard has its operand as soon as x0st3 lands
            for (gst, g) in sorted(pe1_pieces):
                lst = pe1_pieces[(gst, g)]
                for _ in range(4):
                    if not lst:
                        break
                    i, sb_i2 = lst.pop(0)
                    gen_slab_pe1(gst, bc1g[(gst, g)], g * kper + i, i, sb_i2)
                break

            for st in range(NST):
                cols = slice(st * ST, (st + 1) * ST)

                # ---- layer 1: X1 = W1 @ (X0 (x) X0) + b1 ----
                x1ps = [ppc.tile([U, SUB], F32, tag="conv", name=f"x1ps{i}")
                         for i in range(NSUB)]
                for k in range(nk1):
                    # l3 work first: it has no slab dependency, so the PE
                    # chews it while waiting for this iteration's slab DMA
                    emit_l3()
                    had1 = hadp.tile([117, ST], dt_e, tag="had1")
                    nc.vector.tensor_mul(
                        had1[:], x0st3[:, cols],
                        bc1g[(st, k // kper)][:, k % kper, :])
                    for sb_i in range(NSUB):
                        nc.tensor.matmul(
                            x1ps[sb_i][:], w1sb[:, k * U:(k + 1) * U],
                            had1[:, sb_i * SUB:(sb_i + 1) * SUB],
                            start=(k == 0), stop=(k == nk1 - 1),
                        )
                    bc1_drip(st, k)
                x1sb = xsbp.tile([U, ST], dt_e, tag="x1")
                nc.scalar.activation(
                    x1sb[:, 0:SUB], x1ps[0][:],
                    AF.Identity, bias=b1sb[:], scale=1.0)
                for sb_i in range(1, NSUB):
                    nc.scalar.activation(
                        x1sb[:, sb_i * SUB:(sb_i + 1) * SUB], x1ps[sb_i][:],
                        AF.Identity, bias=b1sb[:], scale=1.0)
                # pooled1 contribution: strided d-sum on DVE, dripped later
                # so it never sits between the layer-1 evacs and layer-2 TTs
                def red1(st=st, x1sb=x1sb):
                    nc.vector.tensor_reduce(
                        pooled1[:, st * (ST // D):(st + 1) * (ST // D)],
                        x1sb[:].rearrange("p (b d) -> p b d", d=D),
                        mybir.AxisListType.X, mybir.AluOpType.add)
                l3_pending.append(red1)

                # ---- layer 2: X2 = W2 @ (X0 (x) X1) + b2 ----
                x2ps = [ppc.tile([U, SUB], F32, tag="conv", name=f"x2ps{i}")
                         for i in range(NSUB)]
                for h in range(F):
                    emit_l3()
                    had2 = hadp.tile([U, ST], dt_e, tag="had2")
                    if h == 0:
                        # split per 512-col chunk: each mul chains off its
                        # x1 sub-tile evacuation, so the first L2 matmuls
                        # start as soon as x1ps[0] is evacuated
                        for sb_i in range(NSUB):
                            cs = slice(sb_i * SUB, (sb_i + 1) * SUB)
                            nc.vector.tensor_mul(
                                had2[:, cs], x1sb[:, cs],
                                bc2g[(st, 0)][:, 0, cs])
                            nc.tensor.matmul(
                                x2ps[sb_i][:], w2sb[:, 0:U],
                                had2[:, cs],
                                start=True, stop=False,
                            )
                    else:
                        nc.vector.tensor_mul(
                            had2[:], x1sb[:], bc2g[(st, h // 4)][:, h % 4, :])
                        for sb_i in range(NSUB):
                            nc.tensor.matmul(
                                x2ps[sb_i][:], w2sb[:, h * U:(h + 1) * U],
                                had2[:, sb_i * SUB:(sb_i + 1) * SUB],
                                start=False, stop=(h == F - 1),
                            )
                    bc2_drip(st, h)
                x2sb = xsbp.tile([U, ST], dt_e, tag="x2")
                nc.scalar.activation(
                    x2sb[:, 0:SUB], x2ps[0][:],
                    AF.Identity, bias=b2sb[:], scale=1.0)
                for sb_i in range(1, NSUB):
                    nc.scalar.activation(
                        x2sb[:, sb_i * SUB:(sb_i + 1) * SUB], x2ps[sb_i][:],
                        AF.Identity, bias=b2sb[:], scale=1.0)
                def red2(st=st, x2sb=x2sb):
                    nc.vector.tensor_reduce(
                        pooled2[:, st * (ST // D):(st + 1) * (ST // D)],
                        x2sb[:].rearrange("p (b d) -> p b d", d=D),
                        mybir.AxisListType.X, mybir.AluOpType.add)
                l3_pending.append(red2)

                # ---- layer 3 Gram: G2[m, h, b] = sum_d X2[m,bd] X0[h,bd] ----
                # queue as lazily-emitted pieces, dripped into the next
                # super-tile's conv loops so the PE never starves (HAM warm)
                def queue_l3(st=st, x2sb=x2sb):
                    for s2 in range(ST // D // 2):       # 2 samples / transpose
                        def piece(s2=s2, st=st, x2sb=x2sb):
                            x2t_ps = pptg.tile([U, U], dt_e, tag="tg",
                                               name="x2tps")
                            nc.tensor.transpose(
                                x2t_ps[:],
                                x2sb[:, s2 * 2 * D:(s2 + 1) * 2 * D], id16[:])
                            x2t = l3p.tile([U, U], dt_e, tag="x2t", name="x2t")
                            nc.scalar.activation(x2t[:], x2t_ps[:], AF.Identity)
                            for ls in range(2):
                                b = st * (ST // D) + s2 * 2 + ls
                                g2ps = pptg.tile([U, F], F32, tag="tg",
                                                 name="g2ps")
                                nc.tensor.matmul(
                                    g2ps[:], x2t[ls * D:(ls + 1) * D, :],
                                    x0dt[ls * D:(ls + 1) * D,
                                         b * F:(b + 1) * F],
                                    start=True, stop=True,
                                )
                                nc.scalar.activation(
                                    g2f[:, :, b], g2ps[:], AF.Identity)
                        l3_pending.append(piece)
                queue_l3()
                if st == NST - 1:
                    # w2 is dead after the last conv: pull w3 into its tile
                    nc.gpsimd.dma_start(w3sb[:], w3p_d[:, :])
            while l3_pending:     # flush the last super-tile's layer-3 work
                l3_pending.pop(0)()

            # ---- pooled3 = W3 @ G2 + 64*b3 ----
            if True:
                p3ps = ppbc.tile([U, BC], F32, tag="bcps", name="p3ps")
                for h in range(F):
                    nc.tensor.matmul(
                        p3ps[:], w3sb[:, h * U:(h + 1) * U], g2f[:, h, :],
                        start=(h == 0), stop=(h == F - 1),
                    )
                nc.scalar.activation(
                    pooled3[:], p3ps[:], AF.Identity, bias=b3sb[:], scale=1.0)

                # ---- transpose pooled_i -> [b, o] and store ----
                for i, pl in enumerate((pooled1, pooled2, pooled3)):
                    trp = ppbc.tile([BC, U], F32, tag="bcps", name="trp")
                    nc.tensor.transpose(trp[:], pl[:], id32[:])
                    nc.scalar.activation(
                        outsb[:, i * U:(i + 1) * U], trp[:], AF.Identity)
                nc.sync.dma_start(y_d[:, :], outsb[:])

    nc.compile()
    return nc


def _prep_in_maps(inputs, np_e, sym, offg1):
    X0 = np.asarray(inputs["X_0"], np.float32)
    W1 = np.asarray(inputs["W1"], np.float32)
    b1 = np.asarray(inputs["b1"], np.float32)
    W2 = np.asarray(inputs["W2"], np.float32)
    b2 = np.asarray(inputs["b2"], np.float32)
    W3 = np.asarray(inputs["W3"], np.float32)
    b3 = np.asarray(inputs["b3"], np.float32)

    # [m, h*128+o]
    w2p = W2.reshape(U, F, U).transpose(2, 1, 0).reshape(U, F * U)
    w3p = W3.reshape(U, F, U).transpose(2, 1, 0).reshape(U, F * U)
    es2 = np.zeros((F, F * U), np.float32)
    for h in range(F):
        es2[h, h * U:(h + 1) * U] = 1.0
    es2 = es2.astype(np_e)

    w1r = W1.reshape(U, F, F)                    # [o, h, m]
    if sym:
        # symmetric fold: W1s[o,h,m] = W1[o,h,m] + W1[o,m,h] (h>m), diag as-is
        Msym, Hsym, used = _sym_cover()
        w1s = w1r + w1r.transpose(0, 2, 1)
        for h in range(F):
            w1s[:, h, h] = w1r[:, h, h]
        w1p = np.zeros((117, C1 * U), np.float32)
        es1 = np.zeros((F, C1 * 117), np.float32)
        Hfill = np.where(used, Hsym, 0)
        for k in range(C1):
            for p in range(117):
                if used[k, p]:
                    w1p[p, k * U:(k + 1) * U] = w1s[:, Hsym[k, p], Msym[p]]
                    es1[Hsym[k, p], k * 117 + p] = 1.0
    else:
        # rows p=j*39+m, cols k*128+o -> W1[o, (3k+j)*39+m]
        w1p = np.zeros((117, KG1 * U), np.float32)
        for k in range(KG1):
            for j in range(3):
                w1p[j * F:(j + 1) * F, k * U:(k + 1) * U] = \
                    w1r[:, 3 * k + j, :].T
        es1 = np.zeros((F, KG1 * 117), np.float32)
        for k in range(KG1):
            for p in range(117):
                es1[3 * k + p // F, k * 117 + p] = 1.0
    es1 = es1.astype(np_e)

    shared = {
        "w2p": w2p.astype(np_e),
        "w3p": w3p.astype(np_e),
        "b1c": b1.reshape(U, 1).astype(np.float32),
        "b2c": b2.reshape(U, 1).astype(np.float32),
        "b3c": (D * b3).reshape(U, 1).astype(np.float32),
        "id16": np.eye(U, dtype=np_e),
        "id32": np.eye(U, dtype=np.float32),
    }
    if sym:
        shared["w1ps"] = w1p.astype(np_e)
        if offg1 > 0:
            shared["es1s"] = es1
    else:
        shared["w1p"] = w1p.astype(np_e)
        shared["esel1"] = es1

    in_maps = []
    for c in range(NCORES):
        xs = X0[c * BC:(c + 1) * BC]                         # [128, 39, 64]
        x0cp = xs.transpose(1, 0, 2).reshape(F, BD)          # [h, b*64+d]
        x0dt = xs.transpose(2, 0, 1).reshape(D, BC * F)      # [d, b*39+h]
        x0st = x0cp.reshape(F, NST, ST)
        x0q2 = x0st.transpose(1, 0, 2)                       # [st, h, c]
        m = dict(shared)
        if sym:
            # h-side slab rows gathered on host: [st, g, p, i, c]
            x0q1s = x0cp[np.where(used, Hsym, 0), :]         # [C1, 117, BD]
            x0q1s = x0q1s.reshape(NG1S, KPER, 117, NST, ST)
            x0q1s = np.ascontiguousarray(x0q1s.transpose(3, 0, 2, 1, 4))
            m["x0q1s"] = x0q1s.astype(np_e)
            m["x0mp"] = np.ascontiguousarray(x0cp[Msym, :]).astype(np_e)
        else:
            x0q1 = np.zeros((NST, 3, KG1, ST), np.float32)
            for j in range(3):
                for k in range(KG1):
                    x0q1[:, j, k, :] = x0st[3 * k + j].reshape(NST, ST)
            m["x0q1"] = np.ascontiguousarray(x0q1).astype(np_e)
            m["x0cp"] = x0cp.astype(np_e)
        m["x0dt"] = x0dt.astype(np_e)
        m["x0q2"] = np.ascontiguousarray(x0q2).astype(np_e)
        m["esel2"] = es2
        in_maps.append(m)
    return in_maps


def _run(inputs, trace=False, **kw):
    dt_e, np_e = _dtype_cfg()
    offg2, offg1, sym = _off_cfg()
    key = (dt_e, offg2, offg1, sym)
    if key not in _CACHE:
        _CACHE[key] = _build(dt_e, offg2, offg1, sym)
    nc = _CACHE[key]
    in_maps = _prep_in_maps(inputs, np_e, sym, offg1)
    res = bass_utils.run_bass_kernel_spmd(
        nc, in_maps, core_ids=list(range(NCORES)), trace=trace, **kw)
    y = np.concatenate([r["y"] for r in res.results], axis=0).astype(np.float32)
    return y, res


def kernel(**inputs) -> np.ndarray:
    y, _ = _run(inputs, trace=False)
    return y


# revision 69
# speedup vs baseline: 1.0914x; 1.0508x over previous
"""CIN (xDeepFM Compressed Interaction Network) Bass/Tile kernel for TRN2.

Problem: X_0 [1024, 39, 64]; three CIN layers (units 128 each):
    had_i = outer(X_0, X_i) over channel dims, per (b, d)
    X_{i+1} = W_i @ had_i + b_i            (1x1 conv over channels)
    pooled_i = X_{i+1}.sum(d)
Output: concat(pooled_1..3) -> [1024, 384] fp32.

Strategy (pure data-parallel over batch, 8 cores, 128 samples each):
  * channel-major layout on chip: tensors stored [channels, b*64+d]
  * symmetric layer-1 fold: X0 (x) X0 has 780 unique channel pairs, not
    1521; a multiplicity-weighted static m-pattern (ceil((39-m)/8) rows
    per m, 115 of 117 rows used) covers them in 8 chunks instead of 13,
    with W1 folded as W1[o,h,m]+W1[o,m,h] host-side. Cuts layer-1 PE
    matmuls, DVE multiplies and slab DMA bytes by ~38% each
  * had formation: DVE tensor_mul against 128-row broadcast slabs of X_0
    rows. Layer-1 h-side slabs are host-gathered and loaded as plain fat
    contiguous DMAs; layer-2 slabs arrive by partition-broadcast DMA,
    with 3 of 10 groups per super-tile generated on the PE via one-hot
    selector matmuls dripped between conv matmuls (ScalarE evacuates;
    GPSIMD cannot read PSUM on TRN2)
  * startup: the ST0-critical tensors load via the gpsimd swdge ring
    (software descriptor generation is far cheaper than hwdge), while a
    dummy Pool op gates all non-critical statics on bc2-group-0's
    arrival so they never steal startup DMA bandwidth
  * cross-super-tile slab prefetch EARLY in the h-loop (h=4..14) via a
    dedicated 2-slot bc2pre pool for groups 0/1, keeping the FIFO
    allocation order equal to consumption order; the super-tile junction
    previously stalled ~10us on a tail DMA backlog
  * convs: PE matmuls, k-outer loop accumulating 4 x 512-wide PSUM banks
    per super-tile; layer-2 h=0 hadamard is split per 512-col chunk so
    its matmuls chain off the first x1 evacuation (junction overlap);
    emit_l3 runs at the top of each loop body so the PE chews layer-3
    work while waiting on slab DMA
  * layer 3 never materializes its conv: pooled_3 = W3 @ Gram(X0, X2);
    per 2 samples one PE transpose, then per-sample Gram matmuls; all
    layer-3 pieces drip into the next super-tile's conv loops; W3 loads
    into W2's SBUF tile after the last conv (SBUF is at capacity)
  * elementwise dtype fp16 (DVE 2x mode, ~3e-4 rel err)

Measured on 8 axon TRN2 cores: 434 us HW exec, rel err 3.0e-4
(baseline inherited at 523 us; identical dataflow first measured 1348).
"""

import os
import numpy as np

import concourse.bass as bass
import concourse.bacc as bacc
import concourse.mybir as mybir
import concourse.tile as tile
from concourse import bass_utils

# Walrus's redundant-ldweights elimination (rejects bass-emitted
# InstLdweights as of this toolchain — kept behind an off-by-default flag).
if os.environ.get("BASS_CIN_LDWOPT", "0") == "1" and not getattr(
        bass_utils.run_command, "_cin_ldwopt", False):
    _orig_run_command = bass_utils.run_command

    def _run_command_ldwopt(argv, **kw):
        if isinstance(argv, list):
            argv = ["--enable-ldw-opt=true" if a == "--enable-ldw-opt=false"
                    else a for a in argv]
        return _orig_run_command(argv, **kw)

    _run_command_ldwopt._cin_ldwopt = True
    bass_utils.run_command = _run_command_ldwopt

F32 = mybir.dt.float32
F16 = mybir.dt.float16

B, F, D, U = 1024, 39, 64, 128
NCORES = 8
BC = B // NCORES            # 128 samples per core
BD = BC * D                 # 8192 bd-columns per core
ST = 2048                   # super-tile width (DMA granularity)
NST = BD // ST              # 4
SUB = 512                   # matmul/evac sub-tile width (one PSUM bank)
NSUB = ST // SUB            # 4
SPS = SUB // D              # 8 samples per sub-tile
KG1 = 13                    # layer-1 chunks: 13 x (3 h-values x 39 m) = 117 rows

_CACHE: dict = {}


def _dtype_cfg():
    name = os.environ.get("BASS_CIN_DTYPE", "fp16")
    if name == "fp32":
        return F32, np.float32
    return F16, np.float16


def _off_cfg():
    """How many layer-2 h-groups (of 4) and layer-1 k-groups per super-tile
    are generated on the PE (ones-matmul broadcast + GpSimd/ScalarE
    evacuation) instead of DMA; plus whether the symmetric layer-1 fold
    is enabled (X0 (x) X0 is symmetric: 780 unique pairs covered by 8
    chunks of 117 rows instead of 13)."""
    offg2 = int(os.environ.get("BASS_CIN_OFFG2", "3"))
    offg1 = int(os.environ.get("BASS_CIN_OFFG1", "0"))
    sym = int(os.environ.get("BASS_CIN_SYM", "1"))
    return offg2, offg1, sym


C1 = 8                       # symmetric layer-1 chunks (8 x 117 >= 780 pairs)
KPER = 4                     # chunks per bc1 slab group (fat DMA descriptors)
NG1S = C1 // KPER            # bc1 slab groups per super-tile


def _sym_cover():
    """Multiplicity-weighted m-pattern + pair assignment for the symmetric
    layer-1 fold. Returns (M [117], H [C1, 117], used [C1, 117])."""
    count = [int(np.ceil((F - m) / C1)) for m in range(F)]
    M = list(range(F))
    for extra in range(1, max(count)):
        for m in range(F):
            if count[m] > extra:
                M.append(m)
    M = M + [0] * (117 - len(M))
    H = -np.ones((C1, 117), dtype=np.int64)
    used = np.zeros((C1, 117), dtype=bool)
    rows_of_m: dict = {}
    for p, m in enumerate(M[:115]):
        rows_of_m.setdefault(m, []).append(p)
    for m in range(F):
        hs = list(range(m, F))
        slots = [(c, p) for p in rows_of_m[m] for c in range(C1)]
        for (c, p), h in zip(slots, hs):
            H[c, p] = h
            used[c, p] = True
    return np.array(M), H, used


def _build(dt_e, offg2, offg1, sym) -> bacc.Bacc:
    nc = bacc.Bacc("TRN2", target_bir_lowering=False, debug=False,
                   enable_asserts=False)
    AF = mybir.ActivationFunctionType

    nk1 = C1 if sym else KG1                 # layer-1 chunk count
    kper = KPER if sym else 4                # chunks per bc1 slab group

    es1_d = None
    if sym:
        # x0mp rows: multiplicity-weighted m-pattern (rows 0..38 = X0 direct)
        x0cp_d = nc.dram_tensor("x0mp", [117, BD], dt_e, kind="ExternalInput")
        x0q1_d = nc.dram_tensor("x0q1s", [NST, NG1S, 117, KPER, ST], dt_e,
                                kind="ExternalInput")
        if offg1 > 0:
            es1_d = nc.dram_tensor("es1s", [F, C1 * 117], dt_e,
                                   kind="ExternalInput")
        w1p_d = nc.dram_tensor("w1ps", [117, C1 * U], dt_e,
                               kind="ExternalInput")
    else:
        x0cp_d = nc.dram_tensor("x0cp", [F, BD], dt_e, kind="ExternalInput")
        x0q1_d = nc.dram_tensor("x0q1", [NST, 3, KG1, ST], dt_e,
                                kind="ExternalInput")
        es1_d = nc.dram_tensor("esel1", [F, KG1 * 117], dt_e,
                               kind="ExternalInput")
        w1p_d = nc.dram_tensor("w1p", [117, KG1 * U], dt_e,
                               kind="ExternalInput")
    x0q2_d = nc.dram_tensor("x0q2", [NST, F, ST], dt_e, kind="ExternalInput")
    es2_d = nc.dram_tensor("esel2", [F, F * U], dt_e, kind="ExternalInput")
    x0dt_d = nc.dram_tensor("x0dt", [D, BC * F], dt_e, kind="ExternalInput")
    w2p_d = nc.dram_tensor("w2p", [U, F * U], dt_e, kind="ExternalInput")
    w3p_d = nc.dram_tensor("w3p", [U, F * U], dt_e, kind="ExternalInput")
    b1_d = nc.dram_tensor("b1c", [U, 1], F32, kind="ExternalInput")
    b2_d = nc.dram_tensor("b2c", [U, 1], F32, kind="ExternalInput")
    b3_d = nc.dram_tensor("b3c", [U, 1], F32, kind="ExternalInput")  # 64*b3
    id16_d = nc.dram_tensor("id16", [U, U], dt_e, kind="ExternalInput")
    id32_d = nc.dram_tensor("id32", [U, U], F32, kind="ExternalInput")

    y_d = nc.dram_tensor("y", [BC, 3 * U], F32, kind="ExternalOutput")

    ng1 = (nk1 + kper - 1) // kper           # bc1 groups per super-tile
    ng2 = (F + 3) // 4                       # 10 h-groups (9x4 + 3)
    pe1 = set(list(range(1, ng1, 2))[:offg1])
    pe2 = set(list(range(1, ng2, 2))[:offg2])
    # (measured slower: PE-generating ST0's first bc1 group regressed
    # 444us -> 458us, so it stays disabled)
    pe1_st0: set = set()

    with tile.TileContext(nc) as tc:
        with (
            tc.tile_pool(name="static", bufs=1) as stat,
            tc.tile_pool(name="bc1", bufs=2) as bc1p,
            tc.tile_pool(name="bc2pre", bufs=2) as bc2pre,
            tc.tile_pool(name="bc2", bufs=3) as bc2p,
            tc.tile_pool(name="had", bufs=3) as hadp,
            tc.tile_pool(name="xsb", bufs=1) as xsbp,
            tc.tile_pool(name="l3sb", bufs=2) as l3p,
            tc.tile_pool(name="ps_conv", bufs=4, space="PSUM") as ppc,
            tc.tile_pool(name="ps_tg", bufs=2, space="PSUM") as pptg,
            tc.tile_pool(name="ps_bc", bufs=2, space="PSUM") as ppbc,
        ):
            # ---- ST0-critical loads via the gpsimd swdge ring: descriptor
            # generation there is ~100x cheaper than hwdge (~0.34ns vs
            # ~50ns/desc), so x0st3 lands in ~8us instead of ~35us ----
            x0st3 = stat.tile([117, BD], dt_e)
            if sym:
                nc.gpsimd.dma_start(x0st3[:], x0cp_d[:, :])
            else:
                for j in range(3):
                    nc.gpsimd.dma_start(x0st3[j * F:(j + 1) * F, :],
                                        x0cp_d[:, :])
            w1sb = stat.tile([117, nk1 * U], dt_e)
            nc.gpsimd.dma_start(w1sb[:], w1p_d[:, :])
            b1sb = stat.tile([U, 1], F32)
            nc.gpsimd.dma_start(b1sb[:], b1_d[:, :])
            esel1 = None
            if es1_d is not None:
                esel1 = stat.tile([F, nk1 * 117], dt_e)
                nc.gpsimd.dma_start(esel1[:], es1_d[:, :])

            # ---- global slab-group state (cross-ST prefetch) ----
            bc1g: dict = {}
            bc2g: dict = {}
            pe1_pieces: dict = {}
            pe2_pieces: dict = {}
            dma_rr = [0]

            def ensure_bc1(st, g):
                if st >= NST or g >= ng1 or (st, g) in bc1g:
                    return
                k0 = g * kper
                kcnt = min(kper, nk1 - k0)
                if g in pe1 or (st, g) in pe1_st0:
                    slab = bc1p.tile([117, kper, ST], dt_e, tag="bc1",
                                     name="bc1pe")
                    pe1_pieces[(st, g)] = [(i, sb) for i in range(kcnt)
                                           for sb in range(NSUB)]
                else:
                    slab = bc1p.tile([117, kper, ST], dt_e, tag="bc1",
                                     name="bc1s")
                    if sym:
                        # pre-gathered h-side rows: plain contiguous load.
                        # Mid-kernel prefetches ride the swdge ring (Pool is
                        # idle; keeps bc1 off the hwdge generators that bc2
                        # broadcasts need at super-tile tails). ST0's groups
                        # use hwdge: swdge serializes the bulk startup
                        # transfers behind x0st3 (measured +11us to start)
                        if st == 0:
                            eng = nc.sync if dma_rr[0] % 2 == 0 \
                                else nc.scalar
                            dma_rr[0] += 1
                        else:
                            eng = nc.gpsimd
                        eng.dma_start(slab[:, 0:kcnt, :], x0q1_d[st, g])
                    else:
                        eng = nc.sync if dma_rr[0] % 2 == 0 else nc.scalar
                        for j in range(3):
                            src = x0q1_d[st, j, k0:k0 + kcnt, :] \
                                .partition_broadcast(F)
                            eng.dma_start(
                                slab[j * F:(j + 1) * F, 0:kcnt, :], src)
                        dma_rr[0] += 1
                bc1g[(st, g)] = slab

            def ensure_bc2(st, g):
                if st >= NST or g >= ng2 or (st, g) in bc2g:
                    return
                h0 = g * 4
                hcnt = min(4, F - h0)
                # groups 0/1 live in their own small pool so they can be
                # prefetched EARLY in the previous ST's h-loop (when the DMA
                # queues are idle) without breaking the main pool's
                # allocation-order-equals-consumption-order FIFO discipline
                pool = bc2pre if g < 2 else bc2p
                if g in pe2:
                    slab = pool.tile([U, 4, ST], dt_e, tag="bc2",
                                     name="bc2pe")
                    pe2_pieces[(st, g)] = [(i, sb) for i in range(hcnt)
                                           for sb in range(NSUB)]
                else:
                    slab = pool.tile([U, 4, ST], dt_e, tag="bc2",
                                     name="bc2s")
                    src = x0q2_d[st, h0:h0 + hcnt, :].partition_broadcast(U)
                    eng = nc.sync if dma_rr[0] % 2 == 0 else nc.scalar
                    dma_rr[0] += 1
                    eng.dma_start(slab[:, 0:hcnt, :], src)
                bc2g[(st, g)] = slab

            ensure_bc1(0, 0)
            ensure_bc1(0, 1)
            ensure_bc2(0, 0)
            ensure_bc2(0, 1)

            # w2sb/esel2 are L2-ST0-critical (~needed at t+55us, and the
            # gate only opens ~t+50us): load them ungated
            w2sb = stat.tile([U, F * U], dt_e)
            nc.gpsimd.dma_start(w2sb[:], w2p_d[:, :])
            esel2 = stat.tile([F, F * U], dt_e)
            nc.gpsimd.dma_start(esel2[:], es2_d[:, :])

            # ---- gated loads (gpsimd swdge ring): a dummy Pool op that
            # consumes bc2 group 0 delays every subsequent swdge DMA until
            # the ST0-critical tensors have landed, so they never steal DMA
            # bandwidth from the critical startup path ----
            gate = stat.tile([1, 8], dt_e)
            nc.gpsimd.tensor_copy(gate[:], bc2g[(0, 0)][0:1, 0, 0:8])
            # two stacked copies (partitions 0-63 and 64-127) so per-sample
            # Gram matmuls can match lhsT base_partition for both halves
            x0dt = stat.tile([2 * D, BC * F], dt_e)
            nc.gpsimd.dma_start(x0dt[0:D, :], x0dt_d[:, :])
            nc.gpsimd.dma_start(x0dt[D:2 * D, :], x0dt_d[:, :])
            id16 = stat.tile([U, U], dt_e)
            nc.gpsimd.dma_start(id16[:], id16_d[:, :])
            # w3 reuses w2's tile: its load is issued after the last layer-2
            # conv consumes w2 (see end of the main loop)
            w3sb = w2sb
            b2sb = stat.tile([U, 1], F32)
            nc.gpsimd.dma_start(b2sb[:], b2_d[:, :])
            b3sb = stat.tile([U, 1], F32)
            nc.gpsimd.dma_start(b3sb[:], b3_d[:, :])
            id32 = stat.tile([U, U], F32)
            nc.gpsimd.dma_start(id32[:], id32_d[:, :])

            pooled1 = stat.tile([U, BC], F32)
            pooled2 = stat.tile([U, BC], F32)
            pooled3 = stat.tile([U, BC], F32)
            g2f = stat.tile([U, F, BC], dt_e)           # Gram: [m, h, b]
            outsb = stat.tile([BC, 3 * U], F32)

            # ---- main loop over super-tiles ----
            l3_pending: list = []

            def emit_l3():
                if l3_pending:
                    l3_pending.pop(0)()

            def gen_slab_pe2(st, slab, h, i, sb_i):
                # slab[:, i, sub] <- broadcast of X0[h, sub-cols] to 128
                # rows via selector matmul: esel2[:, h-block].T @ x0block.
                # Evacuations alternate ScalarE/GpSimd so neither FIFO
                # gates the PE through the 2-slot bps pool.
                c0 = st * ST + sb_i * SUB
                bps = ppbc.tile([U, SUB], F32, tag="bcps", name="bps")
                nc.tensor.matmul(
                    bps[:], esel2[:, h * U:(h + 1) * U],
                    x0st3[0:F, c0:c0 + SUB],
                    start=True, stop=True,
                )
                # GPSIMD cannot read PSUM on TRN2: all casts go to ScalarE
                # (Act has headroom; DVE stays exclusively on had-muls)
                dst = slab[:, i, sb_i * SUB:(sb_i + 1) * SUB]
                nc.scalar.activation(dst, bps[:], AF.Identity)

            def gen_slab_pe1(st, slab, k, i, sb_i):
                # one selector matmul builds the whole 117-row chunk:
                # out[p] = X0[3k + p//39]
                c0 = st * ST + sb_i * SUB
                bps = ppbc.tile([117, SUB], F32, tag="bcps", name="bps1")
                nc.tensor.matmul(
                    bps[:], esel1[:, k * 117:(k + 1) * 117],
                    x0st3[0:F, c0:c0 + SUB],
                    start=True, stop=True,
                )
                dst = slab[:, i, sb_i * SUB:(sb_i + 1) * SUB]
                nc.scalar.activation(dst, bps[:], AF.Identity)

            def drip_pe2_pieces(budget):
                for (gst, g) in sorted(pe2_pieces):
                    lst = pe2_pieces[(gst, g)]
                    if lst:
                        for _ in range(budget):
                            if not lst:
                                break
                            i, sb_i2 = lst.pop(0)
                            gen_slab_pe2(gst, bc2g[(gst, g)], g * 4 + i,
                                         i, sb_i2)
                        return True
                return False

            def bc1_drip(st, k):
                ensure_bc1(st, k // kper + 2)
                for (gst, g) in sorted(pe1_pieces):
                    lst = pe1_pieces[(gst, g)]
                    if lst:
                        for _ in range(4):
                            if not lst:
                                break
                            i, sb_i2 = lst.pop(0)
                            gen_slab_pe1(gst, bc1g[(gst, g)], g * kper + i,
                                         i, sb_i2)
                        return
                # no bc1 pieces pending: pre-build pending bc2 pieces
                # (not on ST0, where esel2 is still loading — an early piece
                # matmul would head-of-line-block the in-order PE stream)
                if st > 0:
                    drip_pe2_pieces(3)

            def bc2_drip(st, h):
                ensure_bc2(st, h // 4 + 2)
                # cross-ST prefetch EARLY in the h-loop, when the DMA queues
                # are idle (issuing in the tail piles onto the backlog of
                # this ST's own late bc2 groups and stalls the junction)
                if h == 4:
                    ensure_bc1(st + 1, 0)
                elif h == 6:
                    ensure_bc1(st + 1, 1)
                elif h == 8:
                    ensure_bc2(st + 1, 0)
                elif h == 10:
                    ensure_bc2(st + 1, 1)
                elif h == 30:
                    ensure_bc2(st + 1, 2)
                if h >= F - 3:
                    # keep the Act queue shallow at the super-tile junction
                    # (pending casts would delay the x2 PSUM evacuations);
                    # deferred pieces drip during the next ST's loops
                    return
                drip_pe2_pieces(4)

            # eagerly build the first slab chunk of the PE-generated ST0
            # group so k=0's hadamard has its operand as soon as x0st3 lands
            for (gst, g) in sorted(pe1_pieces):
                lst = pe1_pieces[(gst, g)]
                for _ in range(4):
                    if not lst:
                        break
                    i, sb_i2 = lst.pop(0)
                    gen_slab_pe1(gst, bc1g[(gst, g)], g * kper + i, i, sb_i2)
                break

            for st in range(NST):
                cols = slice(st * ST, (st + 1) * ST)

                # ---- layer 1: X1 = W1 @ (X0 (x) X0) + b1 ----
                x1ps = [ppc.tile([U, SUB], F32, tag="conv", name=f"x1ps{i}")
                         for i in range(NSUB)]
                for k in range(nk1):
                    # l3 work first: it has no slab dependency, so the PE
                    # chews it while waiting for this iteration's slab DMA
                    emit_l3()
                    had1 = hadp.tile([117, ST], dt_e, tag="had1")
                    nc.vector.tensor_mul(
                        had1[:], x0st3[:, cols],
                        bc1g[(st, k // kper)][:, k % kper, :])
                    for sb_i in range(NSUB):
                        nc.tensor.matmul(
                            x1ps[sb_i][:], w1sb[:, k * U:(k + 1) * U],
                            had1[:, sb_i * SUB:(sb_i + 1) * SUB],
                            start=(k == 0), stop=(k == nk1 - 1),
                        )
                    bc1_drip(st, k)
                x1sb = xsbp.tile([U, ST], dt_e, tag="x1")
                nc.scalar.activation(
                    x1sb[:, 0:SUB], x1ps[0][:],
                    AF.Identity, bias=b1sb[:], scale=1.0)
                for sb_i in range(1, NSUB):
                    nc.scalar.activation(
                        x1sb[:, sb_i * SUB:(sb_i + 1) * SUB], x1ps[sb_i][:],
                        AF.Identity, bias=b1sb[:], scale=1.0)
                # pooled1 contribution: strided d-sum on DVE, dripped later
                # so it never sits between the layer-1 evacs and layer-2 TTs
                def red1(st=st, x1sb=x1sb):
                    nc.vector.tensor_reduce(
                        pooled1[:, st * (ST // D):(st + 1) * (ST // D)],
                        x1sb[:].rearrange("p (b d) -> p b d", d=D),
                        mybir.AxisListType.X, mybir.AluOpType.add)
                l3_pending.append(red1)

                # ---- layer 2: X2 = W2 @ (X0 (x) X1) + b2 ----
                x2ps = [ppc.tile([U, SUB], F32, tag="conv", name=f"x2ps{i}")
                         for i in range(NSUB)]
                for h in range(F):
                    emit_l3()
                    had2 = hadp.tile([U, ST], dt_e, tag="had2")
                    if h == 0:
                        # split per 512-col chunk: each mul chains off its
                        # x1 sub-tile evacuation, so the first L2 matmuls
                        # start as soon as x1ps[0] is evacuated
                        for sb_i in range(NSUB):
                            cs = slice(sb_i * SUB, (sb_i + 1) * SUB)
                            nc.vector.tensor_mul(
                                had2[:, cs], x1sb[:, cs],
                                bc2g[(st, 0)][:, 0, cs])
                            nc.tensor.matmul(
                                x2ps[sb_i][:], w2sb[:, 0:U],
                                had2[:, cs],
                                start=True, stop=False,
                            )
                    else:
                        nc.vector.tensor_mul(
                            had2[:], x1sb[:], bc2g[(st, h // 4)][:, h % 4, :])
                        for sb_i in range(NSUB):
                            nc.tensor.matmul(
                                x2ps[sb_i][:], w2sb[:, h * U:(h + 1) * U],
                                had2[:, sb_i * SUB:(sb_i + 1) * SUB],
                                start=False, stop=(h == F - 1),
                            )
                    bc2_drip(st, h)
                x2sb = xsbp.tile([U, ST], dt_e, tag="x2")
                nc.scalar.activation(
                    x2sb[:, 0:SUB], x2ps[0][:],
                    AF.Identity, bias=b2sb[:], scale=1.0)
                for sb_i in range(1, NSUB):
                    nc.scalar.activation(
                        x2sb[:, sb_i * SUB:(sb_i + 1) * SUB], x2ps[sb_i][:],
                        AF.Identity, bias=b2sb[:], scale=1.0)
                def red2(st=st, x2sb=x2sb):
                    nc.vector.tensor_reduce(
                        pooled2[:, st * (ST // D):(st + 1) * (ST // D)],
                        x2sb[:].rearrange("p (b d) -> p b d", d=D),
                        mybir.AxisListType.X, mybir.AluOpType.add)
                l3_pending.append(red2)

                # ---- layer 3 Gram: G2[m, h, b] = sum_d X2[m,bd] X0[h,bd] ----
                # queue as lazily-emitted pieces, dripped into the next
                # super-tile's conv loops so the PE never starves (HAM warm)
                def queue_l3(st=st, x2sb=x2sb):
                    for s2 in range(ST // D // 2):       # 2 samples / transpose
                        def piece(s2=s2, st=st, x2sb=x2sb):
                            x2t_ps = pptg.tile([U, U], dt_e, tag="tg",
                                               name="x2tps")
                            nc.tensor.transpose(
                                x2t_ps[:],
                                x2sb[:, s2 * 2 * D:(s2 + 1) * 2 * D], id16[:])
                            x2t = l3p.tile([U, U], dt_e, tag="x2t", name="x2t")
                            nc.scalar.activation(x2t[:], x2t_ps[:], AF.Identity)
                            for ls in range(2):
                                b = st * (ST // D) + s2 * 2 + ls
                                g2ps = pptg.tile([U, F], F32, tag="tg",
                                                 name="g2ps")
                                nc.tensor.matmul(
                                    g2ps[:], x2t[ls * D:(ls + 1) * D, :],
                                    x0dt[ls * D:(ls + 1) * D,
                                         b * F:(b + 1) * F],
                                    start=True, stop=True,
                                )
                                nc.scalar.activation(
                                    g2f[:, :, b], g2ps[:], AF.Identity)
                        l3_pending.append(piece)
                queue_l3()
                if st == NST - 1:
                    # w2 is dead after the last conv: pull w3 into its tile
                    nc.gpsimd.dma_start(w3sb[:], w3p_d[:, :])
            while l3_pending:     # flush the last super-tile's layer-3 work
                l3_pending.pop(0)()

            # ---- pooled3 = W3 @ G2 + 64*b3 ----
            if True:
                p3ps = ppbc.tile([U, BC], F32, tag="bcps", name="p3ps")
                for h in range(F):
                    nc.tensor.matmul(
                        p3ps[:], w3sb[:, h * U:(h + 1) * U], g2f[:, h, :],
                        start=(h == 0), stop=(h == F - 1),
                    )
                nc.scalar.activation(
                    pooled3[:], p3ps[:], AF.Identity, bias=b3sb[:], scale=1.0)

                # ---- transpose pooled_i -> [b, o] and store ----
                for i, pl in enumerate((pooled1, pooled2, pooled3)):
                    trp = ppbc.tile([BC, U], F32, tag="bcps", name="trp")
                    nc.tensor.transpose(trp[:], pl[:], id32[:])
                    nc.scalar.activation(
                        outsb[:, i * U:(i + 1) * U], trp[:], AF.Identity)
                nc.sync.dma_start(y_d[:, :], outsb[:])

    nc.compile()
    return nc


def _prep_in_maps(inputs, np_e, sym, offg1):
    X0 = np.asarray(inputs["X_0"], np.float32)
    W1 = np.asarray(inputs["W1"], np.float32)
    b1 = np.asarray(inputs["b1"], np.float32)
    W2 = np.asarray(inputs["W2"], np.float32)
    b2 = np.asarray(inputs["b2"], np.float32)
    W3 = np.asarray(inputs["W3"], np.float32)
    b3 = np.asarray(inputs["b3"], np.float32)

    # [m, h*128+o]
    w2p = W2.reshape(U, F, U).transpose(2, 1, 0).reshape(U, F * U)
    w3p = W3.reshape(U, F, U).transpose(2, 1, 0).reshape(U, F * U)
    es2 = np.zeros((F, F * U), np.float32)
    for h in range(F):
        es2[h, h * U:(h + 1) * U] = 1.0
    es2 = es2.astype(np_e)

    w1r = W1.reshape(U, F, F)                    # [o, h, m]
    if sym:
        # symmetric fold: W1s[o,h,m] = W1[o,h,m] + W1[o,m,h] (h>m), diag as-is
        Msym, Hsym, used = _sym_cover()
        w1s = w1r + w1r.transpose(0, 2, 1)
        for h in range(F):
            w1s[:, h, h] = w1r[:, h, h]
        w1p = np.zeros((117, C1 * U), np.float32)
        es1 = np.zeros((F, C1 * 117), np.float32)
        Hfill = np.where(used, Hsym, 0)
        for k in range(C1):
            for p in range(117):
                if used[k, p]:
                    w1p[p, k * U:(k + 1) * U] = w1s[:, Hsym[k, p], Msym[p]]
                    es1[Hsym[k, p], k * 117 + p] = 1.0
    else:
        # rows p=j*39+m, cols k*128+o -> W1[o, (3k+j)*39+m]
        w1p = np.zeros((117, KG1 * U), np.float32)
        for k in range(KG1):
            for j in range(3):
                w1p[j * F:(j + 1) * F, k * U:(k + 1) * U] = \
                    w1r[:, 3 * k + j, :].T
        es1 = np.zeros((F, KG1 * 117), np.float32)
        for k in range(KG1):
            for p in range(117):
                es1[3 * k + p // F, k * 117 + p] = 1.0
    es1 = es1.astype(np_e)

    shared = {
        "w2p": w2p.astype(np_e),
        "w3p": w3p.astype(np_e),
        "b1c": b1.reshape(U, 1).astype(np.float32),
        "b2c": b2.reshape(U, 1).astype(np.float32),
        "b3c": (D * b3).reshape(U, 1).astype(np.float32),
        "id16": np.eye(U, dtype=np_e),
        "id32": np.eye(U, dtype=np.float32),
    }
    if sym:
        shared["w1ps"] = w1p.astype(np_e)
        if offg1 > 0:
            shared["es1s"] = es1
    else:
        shared["w1p"] = w1p.astype(np_e)
        shared["esel1"] = es1

    in_maps = []
    for c in range(NCORES):
        xs = X0[c * BC:(c + 1) * BC]                         # [128, 39, 64]
        x0cp = xs.transpose(1, 0, 2).reshape(F, BD)          # [h, b*64+d]
        x0dt = xs.transpose(2, 0, 1).reshape(D, BC * F)      # [d, b*39+h]
        x0st = x0cp.reshape(F, NST, ST)
        x0q2 = x0st.transpose(1, 0, 2)                       # [st, h, c]
        m = dict(shared)
        if sym:
            # h-side slab rows gathered on host: [st, g, p, i, c]
            x0q1s = x0cp[np.where(used, Hsym, 0), :]         # [C1, 117, BD]
            x0q1s = x0q1s.reshape(NG1S, KPER, 117, NST, ST)
            x0q1s = np.ascontiguousarray(x0q1s.transpose(3, 0, 2, 1, 4))
            m["x0q1s"] = x0q1s.astype(np_e)
            m["x0mp"] = np.ascontiguousarray(x0cp[Msym, :]).astype(np_e)
        else:
            x0q1 = np.zeros((NST, 3, KG1, ST), np.float32)
            for j in range(3):
                for k in range(KG1):
                    x0q1[:, j, k, :] = x0st[3 * k + j].reshape(NST, ST)
            m["x0q1"] = np.ascontiguousarray(x0q1).astype(np_e)
            m["x0cp"] = x0cp.astype(np_e)
        m["x0dt"] = x0dt.astype(np_e)
        m["x0q2"] = np.ascontiguousarray(x0q2).astype(np_e)
        m["esel2"] = es2
        in_maps.append(m)
    return in_maps


def _run(inputs, trace=False, **kw):
    dt_e, np_e = _dtype_cfg()
    offg2, offg1, sym = _off_cfg()
    key = (dt_e, offg2, offg1, sym)
    if key not in _CACHE:
        _CACHE[key] = _build(dt_e, offg2, offg1, sym)
    nc = _CACHE[key]
    in_maps = _prep_in_maps(inputs, np_e, sym, offg1)
    res = bass_utils.run_bass_kernel_spmd(
        nc, in_maps, core_ids=list(range(NCORES)), trace=trace, **kw)
    y = np.concatenate([r["y"] for r in res.results], axis=0).astype(np.float32)
    return y, res


def kernel(**inputs) -> np.ndarray:
    y, _ = _run(inputs, trace=False)
    return y
